# revision 41
# baseline (speedup 1.0000x reference)
"""Log2Quantizer Trainium2 kernel (raw Bass, no Tile).

Math: the reference's sort/std/rank machinery is dead code (bit_token is
unconditionally overwritten with n_bits), so the computation reduces to:
    delta[b,t] = max over (h,c) of x[b,h,t,c]
    out = delta * 2^(round(log2(max(x/delta, 1e-8))))
i.e. snap x/delta to the nearest power of two in log space, rescale by delta.

Bit-trick (no transcendentals), fp32-exact up to 1-ulp boundary flips:
    q   = x * (sqrt2/delta)                  (M1, per-token scalar mult)
    p2  = bitcast_f32(bits(q) & 0x7F800000)  # 2^floor(log2 q) = 2^(k+1)
    out = p2 * delta                         # (M2) fp32 mult by 2^(k+1), exact
round(log2(x/delta)) = floor(log2(x*sqrt2/delta)); the sqrt2 pre-scale
implements round-half-up in log space, and the extra factor 2 folded into it
shifts the exponent so the final scale is plain delta. x==0 -> q=0 -> out=0.

Sharding: data-parallel over batch dim b (8 rows -> 8 cores), no comms.

v9 schedule (v1 baseline 98.5us was DVE-bound at 77us busy):
  - bf16 stores (rel-err ~1e-3 vs 2e-2 gate) halve store traffic.
  - single DMA queue on the SP ring: all loads first (full bandwidth, no
    round-robin with stores), then stores gated on M2 completion.
  - chunks tapered at BOTH ends [128,128,256,512*6,256,128,128]: short
    first-load ramp, short serial tail after the last load.
  - per-chunk slice split balances DVE (0.46us/slice) vs ACT (0.92us/slice):
    DVE: reduce, recip, inv2, M1/M2 share, the AND; ACT: M1/M2 share.
    The AND of chunk ci runs at the start of DVE iteration ci+1 (after ACT's
    M1 share lands) and ACT runs M1(ci+1) before M2(ci) — both break
    cross-engine stall cycles.
  - ACT activation table pre-warmed with a dummy Copy before any real work.
Buffers/partition: xt 96KB + qt 4x12KB + wt 4x6KB = 168KB < ~208KB usable.
All cross-buffer, no in-place: M1 xt->qt, AND qt->xt, M2 xt->wt.
"""

from contextlib import ExitStack

import numpy as np

import concourse.bass as bass
import concourse.mybir as mybir
from concourse.bass_utils import run_bass_kernel_spmd

B, H, T, C = 8, 12, 4096, 64
N_CORES = 8
P = 128
CHUNKS = [256] + [512] * 7 + [128, 128]   # sum = T
NQ = 3           # qt scratch slots

SQRT2 = 1.4142135623730951
EXP_MASK = 0x7F800000

_nc_cache = {}


def _plan():
    """Per-chunk geometry, slice assignments, cumulative sem-inc indices."""
    n = len(CHUNKS)
    offs, g = [], []
    o = 0
    for tc in CHUNKS:
        offs.append(o)
        o += tc
        g.append(tc // P)
    # slice split per chunk (measured: DVE slice ~0.62us, ACT slice ~0.92us;
    # DVE also carries reduce+AND ~5.3us/chunk):
    # g=4: M1 ACT{0,1,2}/DVE{3}, M2 all ACT   -> DVE 5.9 vs ACT 6.4 per chunk
    # g=2: M1 ACT{0}/DVE{1},     M2 all ACT
    # g=1: everything on DVE (short serial tail, avoid cross-engine hops)
    m1_act, m1_dve, m2_act, m2_dve = [], [], [], []
    for ci, gi in enumerate(g):
        s = list(range(gi))
        na = {4: 3, 2: 1, 1: 0}[gi]
        m1_act.append(s[:na]); m1_dve.append(s[na:])
        if gi == 1:
            m2_act.append([]); m2_dve.append(s)
        else:
            m2_act.append(s); m2_dve.append([])
    # DVE inc sequence per iteration ci:
    #   reduce, recip, inv2, M1own(ci)..., then [AND(ci-1), M2own(ci-1)]:
    # the deferred AND sits at the END of the iteration so ACT's M1 share
    # of ci-1 has the whole reduce phase to land (no DVE stall), and the
    # next iteration's reduce never waits on ACT
    idx_inv2 = [0] * n    # dve_sem value once inv2(ci) done
    idx_m1own = [0] * n   # dve_sem value once all DVE M1 slices of ci done
    idx_and = [0] * n     # dve_sem value once AND(ci) done
    idx_m2own = [0] * n   # dve_sem value once DVE M2 slices of ci done
    cum = 0
    for ci in range(n):
        cum += 3
        idx_inv2[ci] = cum
        cum += len(m1_dve[ci])
        idx_m1own[ci] = cum
        if ci >= 1:
            cum += 1
            idx_and[ci - 1] = cum
            cum += len(m2_dve[ci - 1])
            idx_m2own[ci - 1] = cum
    cum += 1
    idx_and[n - 1] = cum
    cum += len(m2_dve[n - 1])
    idx_m2own[n - 1] = cum
    # act_sem: one inc per chunk with ACT M1 slices (after the last slice)
    cum_m1act = []
    a = 0
    for ci in range(n):
        if m1_act[ci]:
            a += 1
        cum_m1act.append(a)
    # m2act_sem: one inc per chunk with ACT M2 slices
    cum_m2act = []
    a = 0
    for ci in range(n):
        if m2_act[ci]:
            a += 1
        cum_m2act.append(a)
    return dict(
        offs=offs, g=g,
        m1_act=m1_act, m1_dve=m1_dve, m2_act=m2_act, m2_dve=m2_dve,
        idx_inv2=idx_inv2, idx_m1own=idx_m1own, idx_and=idx_and,
        idx_m2own=idx_m2own, cum_m1act=cum_m1act, cum_m2act=cum_m2act,
    )


def _build_nc():
    if "nc" in _nc_cache:
        return _nc_cache["nc"]
    f32 = mybir.dt.float32
    i32 = mybir.dt.int32
    bf16 = mybir.dt.bfloat16
    OP = mybir.AluOpType
    Copy = mybir.ActivationFunctionType.Copy

    pl = _plan()
    offs, g = pl["offs"], pl["g"]
    n_chunks = len(CHUNKS)
    GMAX = max(g)
    FREE = H * GMAX * C

    nc = bass.Bass()
    x_in = nc.declare_dram_parameter("x", [H, T, C], f32, isOutput=False)
    y_out = nc.declare_dram_parameter("y", [H, T, C], bf16, isOutput=True)

    def src_ap(ci):
        return x_in[:, offs[ci] : offs[ci] + CHUNKS[ci], :].rearrange(
            "h (p q) c -> p h (q c)", p=P
        )

    def dst_ap(ci):
        return y_out[:, offs[ci] : offs[ci] + CHUNKS[ci], :].rearrange(
            "h (p q) c -> p h (q c)", p=P
        )

    with ExitStack() as ctx:
        xt = [
            ctx.enter_context(nc.sbuf_tensor(f"xt{j}", [P, H * g[j] * C], f32))
            for j in range(n_chunks)
        ]
        qt = [
            ctx.enter_context(nc.sbuf_tensor(f"qt{j}", [P, FREE], f32))
            for j in range(NQ)
        ]
        # one wt slot per chunk (sized per chunk): no compute ever waits on
        # a store completing, and stores need no semaphores at all (the
        # end-of-block drain covers the last DMAs)
        wt = [
            ctx.enter_context(nc.sbuf_tensor(f"wt{j}", [P, H * g[j] * C], bf16))
            for j in range(n_chunks)
        ]
        delta = [
            ctx.enter_context(nc.sbuf_tensor(f"delta{j}", [P, g[j]], f32))
            for j in range(n_chunks)
        ]
        inv2 = [
            ctx.enter_context(nc.sbuf_tensor(f"inv2_{j}", [P, g[j]], f32))
            for j in range(n_chunks)
        ]
        warm = ctx.enter_context(nc.sbuf_tensor("warm", [P, 1], f32))

        load_sem = [
            ctx.enter_context(nc.semaphore(f"load{j}")) for j in range(n_chunks)
        ]
        dve_sem = ctx.enter_context(nc.semaphore("dve_sem"))
        act_sem = ctx.enter_context(nc.semaphore("act_sem"))     # M1 groups
        m2a_sem = ctx.enter_context(nc.semaphore("m2a_sem"))     # ACT M2 groups
        warm_sem = ctx.enter_context(nc.semaphore("warm_sem"))

        block = ctx.enter_context(nc.Block())

        def v4(buf, ci):
            return buf[:, : H * g[ci] * C].rearrange(
                "p (h q c) -> p h q c", h=H, c=C
            )

        @block.sync
        def _(sync):
            # single DMA queue: loads first at full bandwidth, stores behind
            for ci in range(n_chunks):
                sync.dma_start(
                    out=xt[ci][:], in_=src_ap(ci)
                ).then_inc(load_sem[ci], 16)
            for cj in range(n_chunks - 2):
                if pl["m2_act"][cj]:
                    sync.wait_ge(m2a_sem, pl["cum_m2act"][cj])
                sync.wait_ge(dve_sem, pl["idx_m2own"][cj])
                # reuse the chunk's load sem for store completion (the load
                # finished long before the store can issue; nothing races)
                sync.dma_start(out=dst_ap(cj), in_=wt[cj][:]).then_inc(
                    load_sem[cj], 16
                )

        @block.vector
        def _(vector):
            vector.memset(warm[:], 1.0).then_inc(warm_sem, 1)

            def do_and_m2own(cj):
                # AND: p2 = bits(q) & EXP_MASK, qt -> xt (xt dead after M1);
                # needs ACT's M1 share (act_sem) and own M1 (dve_sem fence)
                vector.wait_ge(dve_sem, pl["idx_m1own"][cj])
                vector.wait_ge(act_sem, pl["cum_m1act"][cj])
                vector.tensor_scalar(
                    out=xt[cj][:].bitcast(i32),
                    in0=qt[cj % NQ][:, : H * g[cj] * C].bitcast(i32),
                    scalar1=EXP_MASK,
                    scalar2=None,
                    op0=OP.bitwise_and,
                ).then_inc(dve_sem, 1)
                if pl["m2_dve"][cj]:
                    vector.wait_ge(dve_sem, pl["idx_and"][cj])
                    xt4 = v4(xt[cj][:], cj)
                    wt4 = v4(wt[cj][:], cj)
                    for s in pl["m2_dve"][cj]:
                        # DVE's M2 share: out = p2 * delta, bf16 out
                        vector.tensor_scalar_mul(
                            wt4[:, :, s, :], xt4[:, :, s, :],
                            delta[cj][:, s : s + 1],
                        ).then_inc(dve_sem, 1)

            for ci in range(n_chunks):
                xt4 = v4(xt[ci][:], ci)
                qt4 = v4(qt[ci % NQ][:], ci)
                vector.wait_ge(load_sem[ci], 16)
                vector.reduce_max(
                    out=delta[ci][:],
                    in_=xt4.transpose([0, 2, 1, 3]),
                    axis=mybir.AxisListType.XY,
                ).then_inc(dve_sem, 1)
                vector.wait_ge(dve_sem, pl["idx_inv2"][ci] - 2)
                vector.reciprocal(inv2[ci][:], delta[ci][:]).then_inc(dve_sem, 1)
                vector.wait_ge(dve_sem, pl["idx_inv2"][ci] - 1)
                vector.tensor_scalar_mul(
                    inv2[ci][:], inv2[ci][:], SQRT2
                ).then_inc(dve_sem, 1)
                vector.wait_ge(dve_sem, pl["idx_inv2"][ci])
                for s in pl["m1_dve"][ci]:
                    # M1: q = x * inv2 ([128,1] per-token scalar, 2x mode)
                    vector.tensor_scalar_mul(
                        qt4[:, :, s, :], xt4[:, :, s, :], inv2[ci][:, s : s + 1]
                    ).then_inc(dve_sem, 1)
                if ci >= 1:
                    do_and_m2own(ci - 1)
            do_and_m2own(n_chunks - 1)

        @block.scalar
        def _(scalar):
            # pre-warm the activation table off the critical path
            scalar.wait_ge(warm_sem, 1)
            scalar.activation(out=warm[:], in_=warm[:], func=Copy)

            def do_m2act(cj):
                if not pl["m2_act"][cj]:
                    return
                xt4 = v4(xt[cj][:], cj)
                wt4 = v4(wt[cj][:], cj)
                scalar.wait_ge(dve_sem, pl["idx_and"][cj])
                for k, s in enumerate(pl["m2_act"][cj]):
                    # ACT's M2 share: out = p2 * delta, bf16 out
                    inst = scalar.activation(
                        out=wt4[:, :, s, :],
                        in_=xt4[:, :, s, :],
                        func=Copy,
                        scale=delta[cj][:, s : s + 1],
                    )
                    if k == len(pl["m2_act"][cj]) - 1:
                        inst.then_inc(m2a_sem, 1)

            for ci in range(n_chunks):
                if pl["m1_act"][ci]:
                    xt4 = v4(xt[ci][:], ci)
                    qt4 = v4(qt[ci % NQ][:], ci)
                    scalar.wait_ge(dve_sem, pl["idx_inv2"][ci])
                    for k, s in enumerate(pl["m1_act"][ci]):
                        # ACT's M1 share via activation Copy, scale=inv2
                        inst = scalar.activation(
                            out=qt4[:, :, s, :],
                            in_=xt4[:, :, s, :],
                            func=Copy,
                            scale=inv2[ci][:, s : s + 1],
                        )
                        if k == len(pl["m1_act"][ci]) - 1:
                            inst.then_inc(act_sem, 1)
                if ci >= 1:
                    do_m2act(ci - 1)
            do_m2act(n_chunks - 1)
            # tail stores ride the idle ACT ring so they start the moment
            # the last chunks' compute lands (no SP-ring residue in front)
            for cj in (n_chunks - 2, n_chunks - 1):
                if pl["m2_act"][cj]:
                    scalar.wait_ge(m2a_sem, pl["cum_m2act"][cj])
                scalar.wait_ge(dve_sem, pl["idx_m2own"][cj])
                scalar.dma_start(out=dst_ap(cj), in_=wt[cj][:]).then_inc(
                    load_sem[cj], 16
                )

    _nc_cache["nc"] = nc
    return nc


def kernel(x: np.ndarray) -> np.ndarray:
    assert x.shape == (B, H, T, C) and x.dtype == np.float32
    nc = _build_nc()
    in_maps = [{"x": np.ascontiguousarray(x[i])} for i in range(N_CORES)]
    res = run_bass_kernel_spmd(nc, in_maps, list(range(N_CORES)))
    out = np.stack(
        [res.results[i]["y"].astype(np.float32) for i in range(N_CORES)], axis=0
    )
    return out
